# revision 63
# baseline (speedup 1.0000x reference)
"""Trainium2 Bass kernel for conditional-adjustment conv (CAConv), fp16.

Per sample b: h = relu(c[b] @ mlp_w1 + mlp_b1); adj = h @ mlp_w2 + mlp_b2;
w[b] = conv_w + adj.reshape(Co,Ci,3,3); out[b] = conv2d(x[b], w[b], pad=1) + conv_b.

Sharding: data-parallel over batch, 4 samples per core on 8 cores (SPMD).

All heavy matmuls in fp16 (full PE rate); psum accumulation stays fp32, so
rel err ~5e-4 << the 2e-2 budget. The host pre-casts padded x and the packed
w2 to fp16 and the kernel returns fp16 output (halves HBM traffic both
ways); the host casts back to fp32.

Per-core device kernel:
  Stage A (weight gen): four col-group fp32 matmuls (M=32 via
  zero-padded w1', tile_position=(0,32g)) compute the MLP hidden state
  for all 4 samples directly as a [128, 32] psum tile; one DVE fused
  add-bias+relu produces hT8 fp16 (col m = sample m%4, replicated at
  partition offsets 0/32/64/96 to match the packed w2). w2 is
  host-permuted to (ci, t, co) column order, packed in 4 k-groups
  [128, 9216] (group g = ci 16g..16g+16, rows 32g..32g+17), with
  mlp_b2 + conv_w folded into ones-row 16. For each 512-col chunk, 4
  matmuls with tile_position=(32g, 32g) write M=32 rows each so one
  psum tile [128, 512] is fully covered -> full-width DVE/ACT copies
  (fp32->fp16) into the partition-grouped adj4 [128, 9216] (row
  32g + 4r + b = sample b, ci group g, replica r).
  Weight placement is a two-hop scatter: (1) a 64-descriptor DMA per
  sample (partition-stride-32 source) into the compact staging tile
  wst[ci + 64*half, (t, co)]; (2) a same-partition strided DVE/ACT copy
  fans each half out onto the diagonal blocks of the per-pair
  block-diag tile wblk[ci + 64*half, t*128 + 64*half + co] (off-diag
  zeros from a DVE memset). This avoids the 576 tiny 128B descriptors
  per sample a direct scatter would need (measured ~16us of DMA-engine
  serialization gating conv start).
  Stage B (conv): host-padded fp16 x (130x130) for a sample pair lives
  as [ci(2 samples), h, w] across 128 partitions. Chunk-outer/
  tap-inner: each output chunk po[128, 512] (2 samples x 64 co
  partitions; 4 h-rows x 128 w free) accumulates its 9 shift-tap K=128
  fp16 matmuls back-to-back at the PE's 1 col/cycle peak (~218ns/MM),
  then its bias-copy (alternating DVE/ACT) fires immediately. Psum
  banks rotate through an explicit 8-tag round-robin ring (the pool's
  own slot picker reuses banks ~2 tiles apart, each reuse a ~1.3us PE
  stall on the copy drain). One output DMA per 16 h-rows; the final
  group's store is split so the tail is short. Plain junk matmuls pad
  the PE between stage A and conv so the HAM clock gate stays at
  full rate (col-tiled matmuls don't register as PE-busy to it).

  All loads ride the SP HWDGE queue (Q1) in consumption order (cst,
  w2 in graduated chunks, the 19-row xp0 head, pair-0 hop1 scatters,
  then the x backlog, then pair-1 scatters); output stores use the ACT
  queue (Q10). The arbiter gives Q1 strict priority, so anything
  parked on other queues starves while Q1 streams — pair-1's scatters
  exploit this deliberately (needed ~45us later).

  HAM (PE clock gate) discipline, measured on silicon: the PE boots at
  4/8 duty (1.2GHz) and un-throttles only after ~3.4us of sustained
  FULL-ARRAY activity; tile-positioned quartets and small (K=17/M=32)
  matmuls are invisible to BOTH monitors, so the original quartet
  stage A ran entirely cold and conv paid a ~4us cold ramp. Stage A's
  body is therefore a block-diagonal ht8b [128,128] stationary driving
  ONE plain K=128 matmul per 512-col chunk (same math as the quartet,
  one instruction, HAM-visible), and ~24 plain junk matmuls bridge the
  scatter/fanout hole so the MID monitor (re-throttles after ~3.4us
  without qualifying activity) never fires before conv.

  Stage A's tail is copy-chain limited: each chunk's psum->adj4 copy
  is split in half across DVE and ACT running simultaneously
  (whole-copy alternation paced the body at ~460-690ns/chunk).
  hop1 scatter descriptors (64 x 1152B, SBUF->SBUF) serialize on a
  small engine set at ~2us/DMA regardless of queue: pair-0's two
  scatters run alone on Q1 at the body's end, pair-1's trail the x
  backlog, and their fanouts are emitted mid-conv-pair-0 so they never
  head-block the conv bias-copies.

  Measured: 158.8us (HW exec) in the chip's fast state (conv matmuls
  at 217ns/512-col = full 2.4GHz streaming rate, no HAM events
  mid-kernel); the board alternates run-to-run into a power-throttled
  state (264ns spacing, everything including DMA ~21% slower) where
  the same kernel measures ~190us. Conv start at ~25.4us
  (fast-equivalent) vs 28.1 for the staged baseline; remaining fixed
  overhead: ~6us NEFF prologue (excluded from exec_time), ~3us first-
  DMA cold latency, ~6.5us w2 stream (gates stage A), ~5us scatter+
  fanout, ~10us teardown (full semaphore-file reset + drains).
"""

import sys

if "/opt/trn_rl_repo" not in sys.path:
    sys.path.insert(0, "/opt/trn_rl_repo")

import numpy as np

B = 32
NCORES = 8
BPC = B // NCORES          # samples per core = 4
PAIRS = BPC // 2           # sample pairs per core = 2
CIN = COUT = 64
H = W = 128
HP = WP = 130              # padded dims
KH = KW = 3
NT = KH * KW               # taps = 9
CL = 8                     # c length
CL1 = CL + 1               # + ones row
MH = 16                    # mlp hidden
K2 = MH + 1                # mlp hidden + ones row
WTOT = NT * CIN * COUT     # 36864 weights per sample
GCOL = WTOT // 4           # 9216 cols per packed w2 group
XCH = 5                    # x chunks per pair
XCHE = (HP * WP) // XCH    # 3380 elems per chunk (26 padded rows)

_CACHE = {}


def _build():
    import concourse.bass as bass
    import concourse.mybir as mybir
    import concourse.tile as tile
    from concourse import bacc
    from concourse.tile_rust import add_dep_helper

    f32 = mybir.dt.float32
    f16 = mybir.dt.float16
    AF = mybir.ActivationFunctionType

    nc = bacc.Bacc("TRN2", target_bir_lowering=False, debug=False)

    xs_d = nc.dram_tensor("xsp", [BPC, CIN, HP * WP], f16, kind="ExternalInput")
    w2_d = nc.dram_tensor("w2p", [128, GCOL], f16, kind="ExternalInput")
    cst_d = nc.dram_tensor("cst", [128, 66], f32, kind="ExternalInput")
    out_d = nc.dram_tensor("out", [BPC, COUT, H, W], f16, kind="ExternalOutput")

    with tile.TileContext(nc) as tc:
        with (
            tc.tile_pool(name="consts", bufs=1) as consts,
            tc.tile_pool(name="adjpool", bufs=1) as adjpool,
            tc.tile_pool(name="xpool", bufs=2) as xpool,
            tc.tile_pool(name="opool", bufs=8) as opool,
            tc.tile_pool(name="pspool", bufs=1, space=bass.MemorySpace.PSUM) as ps,
        ):
            # ---- consts then packed w2 on the SP queue (gate stage A);
            # GPSIMD is avoided entirely: its DSPs take ~10us of ucode
            # load/drain before their first op ----
            # cst: cols 0-31 c'T tiled 8x (rows 0-8, ones row 8, col m =
            # sample m%4), 32-63 w1' zero-padded to M=32 (rows 0-8),
            # 64 b1 tiled at partition offsets 0/32/64/96, 65 conv_b x2
            cst = consts.tile([128, 66], f32)
            nc.sync.dma_start(out=cst[:], in_=cst_d.ap())
            # w2 128-row packed (rows 32g..32g+17 = group g): 2.36MB vs
            # 1.25MB dense, but a dense [17, *] load has only 17
            # descriptors per DMA and the engines assign per-descriptor
            # round-robin within a DMA — measured: the whole dense load
            # serialized onto ONE engine at ~55us. 128 descriptors/DMA
            # spread over all 16 engines beat the byte savings.
            w2s = consts.tile([128, GCOL], f16, name="w2s")
            # graduated chunks: small leading chunks so stage A starts
            # early and never sees a multi-us burst wait; larger tail
            # chunks keep the issue count down (each dma_start costs
            # ~0.6us of SP time)
            w2cuts = [0, 512, 1536, 3072, 4608, 6912, GCOL]
            for c0, c1 in zip(w2cuts, w2cuts[1:]):
                nc.sync.dma_start(
                    out=w2s[:, c0:c1], in_=w2_d.ap()[:, c0:c1]
                )
            ct_sb = cst[0:CL1, 0:32]
            w1_sb = cst[0:CL1, 32:64]
            b1_sb = cst[:, 64:65]
            cb_sb = cst[:, 65:66]

            # psum bank ring: explicit round-robin via 8 single-buffer
            # tags — the pool's own slot picker reuses banks as little
            # as 1 tile apart, each reuse a ~1.3us PE stall on the copy
            # drain.
            psk = [0]

            def ptile(shape, name=None):
                t = ps.tile(shape, f32, tag=f"ps{psk[0] % 8}", bufs=1, name=name)
                psk[0] += 1
                return t

            # ---- stage A head: hT8 via one matmul quartet, then fused
            # bias+relu into the DIAGONAL blocks of ht8b [128, 128]
            # (zeros elsewhere). ht8b block-diag makes the stage A body a
            # PLAIN K=128 matmul per 512-col chunk: one LDW+MM instead of
            # a 4x tile_position quartet, and (critically) plain full-K
            # matmuls register as "busy" to the PE_HAM clock gate, so the
            # PE un-throttles 4/8 -> 8/8 DURING stage A instead of 3.9us
            # into conv. (Measured: tile-positioned quartets are invisible
            # to both HAM monitors — stage A ran entirely at 1.2GHz and
            # the MID monitor even re-throttled mid-kernel.) ----
            ph32 = ptile([128, 32], name="ph32")
            for g in range(4):
                nc.tensor.matmul(
                    ph32[32 * g : 32 * g + 32, :], w1_sb, ct_sb,
                    start=True, stop=True, tile_position=(0, 32 * g),
                )
            ht8b = consts.tile([128, 128], f16, name="ht8b")
            nc.vector.memset(ht8b[:], 0.0)
            for g in range(4):
                nc.vector.tensor_scalar(
                    out=ht8b[32 * g : 32 * g + K2, 32 * g : 32 * g + 32],
                    in0=ph32[32 * g : 32 * g + K2, :],
                    scalar1=cst[32 * g : 32 * g + K2, 64:65], scalar2=0.0,
                    op0=mybir.AluOpType.add, op1=mybir.AluOpType.max,
                )

            # early junk burst: plain full-K matmuls fill the w2-DMA wait
            # window right after ht8b is ready, so the HAM's SHORT monitor
            # fires ~3.4us in (measured: deterministic at ~14us) instead
            # of the run-variable 16-24us when warm-up relied on the
            # body's spaced matmuls alone.
            for w in range(100, 107):
                pw = ptile([128, 512], name=f"warm{w}")
                nc.tensor.matmul(
                    pw[:], ht8b[:], w2s[:, 0:512], start=True, stop=True
                )

            # ---- bulk x loads, all on Q1 (SP). The DMA arbiter gives
            # Q1 strict priority, so parking x on another queue starves
            # that queue instead of helping; the lever that works is
            # ORDER within the Q1 ring: only the two xp0 chunks conv
            # needs first go ahead of the hop1 scatters, the rest after.
            xps = []
            for p in range(PAIRS):
                xp = xpool.tile([128, HP * WP], f16, name=f"xp{p}", tag="xp")
                xps.append(xp)

            # custom row cuts: a small head chunk (rows 0-18, all that
            # conv group g=0 reads) lands fast right behind w2 on Q1; the
            # rest follows the hop1 scatters and stays far ahead of the
            # conv's row consumption.
            XCUTS = [0, 2470, 6110, 9750, 13390, HP * WP]

            def load_x(p, ks, after=()):
                for k in ks:
                    e0, e1 = XCUTS[k], XCUTS[k + 1]
                    inst = nc.sync.dma_start(
                        out=xps[p][:, e0:e1],
                        in_=xs_d.ap()[2 * p : 2 * p + 2, :, e0:e1],
                    )
                    for a in after:
                        add_dep_helper(
                            inst.ins, a.ins, sync=True, reason="x after hop1"
                        )

            load_x(0, range(1))

            # per-pair block-diag weights; off-diag zeros via DVE memset
            # (DVE is idle early; GPSIMD wouldn't wake up for ~10us).
            # Emitted after the hT8 op so they don't head-block it in
            # the DVE FIFO.
            wblk = []
            for p in range(PAIRS):
                wb = consts.tile([128, NT * 128], f16, name=f"wblk{p}", tag=f"wblk{p}")
                nc.vector.memset(wb[:], 0.0)
                wblk.append(wb)

            # ---- stage A body: adj4[32g + 4r + b, c] = sample b's weight
            # for flat col 9216g + c ((ci,t,co) order), r = 0..7 replicas ----
            adj4 = adjpool.tile([128, GCOL], f16, name="adj4")
            for m in range(GCOL // 512):
                j = m * 512
                pa = ptile([128, 512])
                nc.tensor.matmul(
                    pa[:], ht8b[:], w2s[:, j : j + 512], start=True, stop=True
                )
                # whole-512 copies alternating DVE/ACT: 9 ops per engine
                # amortizes the ~200-300ns per-op issue+sem overhead
                # better than half-splits (18 ops/engine), keeping the
                # ring's copy drain close to the w2 stream's 360ns/chunk
                # pace. Only the final chunk (critical path to hop1)
                # splits across both engines.
                if m == GCOL // 512 - 1:
                    nc.vector.tensor_copy(adj4[:, j : j + 256], pa[:, 0:256])
                    nc.scalar.copy(adj4[:, j + 256 : j + 512], pa[:, 256:512])
                elif m % 2 == 0:
                    nc.vector.tensor_copy(adj4[:, j : j + 512], pa[:])
                else:
                    nc.scalar.copy(adj4[:, j : j + 512], pa[:])

            # two-hop weight placement (see module docstring)
            wst = []
            for p in range(PAIRS):
                wstp = consts.tile([128, NT * 64], f16, name=f"wst{p}", tag=f"wst{p}")
                wst.append(wstp)
            # hop1: each dma_start costs ~0.6us of ENGINE time (flat,
            # regardless of descriptor count), so the four scatters are
            # spread across SP/SP/DVE/ACT to issue in parallel the moment
            # adj4's last copy retires.
            def hop1(b, eng, split=False):
                p, half = divmod(b, 2)
                q = half * 64
                r = 9 * b
                # read sample b's replica r=2b (row 32g + 4r + b = 32g+9b):
                # source partitions 9b, 9b+32, .. land on a DISJOINT DMA
                # engine quartet per sample (engine ~ partition//8), so
                # concurrent scatters don't contend. A DMA's 64
                # descriptors process serially (~60ns each => ~3.8us), so
                # the conv-gating pair-0 scatters additionally split by
                # group pairs — with the replica offsets every sub-DMA
                # owns its own engine pair and the transfers quarter.
                if not split:
                    return eng.dma_start(
                        out=wst[p][q : q + 64, :], in_=adj4[r : 128 : 32, :]
                    )
                eng.dma_start(
                    out=wst[p][q : q + 32, :], in_=adj4[r : 64 : 32, :]
                )
                return eng.dma_start(
                    out=wst[p][q + 32 : q + 64, :],
                    in_=adj4[64 + r : 128 : 32, :],
                )

            def fanout(b, eng):
                p, half = divmod(b, 2)
                q = half * 64
                dst = wblk[p][q : q + 64, :].rearrange(
                    "p (t co) -> p t co", co=128
                )[:, :, q : q + 64]
                src = wst[p][q : q + 64, :].rearrange("p (t co) -> p t co", co=64)
                if eng is nc.vector:
                    eng.tensor_copy(dst, src)
                else:
                    eng.copy(dst, src)

            # junk bridge over the hop1/fanout hole: plain full-K matmuls
            # (results never read) hold the HAM's busy monitor so the
            # clock stays 8/8 into conv. Sized to OVERLAP the whole
            # scatter+fanout handoff (~8us): an undersized bridge leaves
            # a >3.4us idle gap where the MID monitor re-throttles and
            # conv restarts cold (~3us penalty, run-to-run luck); the
            # overshoot cost is bounded by conv waiting on the last junk
            # (~equal to the handoff length either way).
            for w in range(26):
                pw = ptile([128, 512], name=f"warm{w}")
                nc.tensor.matmul(
                    pw[:], ht8b[:], w2s[:, 0:512], start=True, stop=True
                )

            # pair-0 scatters first: hop1(0) on SP/Q1, hop1(1) on ACT/Q10
            # (empty until the first conv store) — disjoint queues AND,
            # via the replica trick, disjoint engine quartets, so their
            # issues and transfers run fully in parallel.
            h_a = hop1(0, nc.sync, split=True)
            h_b = hop1(1, nc.scalar, split=True)
            fanout(0, nc.vector)
            fanout(1, nc.scalar)
            # fanout(2)/(3) are emitted mid-way through conv pair 0 (see
            # below) so they don't head-block the conv's DVE/ACT
            # bias-copies.
            # x backlog after the pair-0 scatters (Q1 FIFO order).
            load_x(0, range(1, XCH), after=(h_b,))
            load_x(1, range(XCH), after=(h_b,))
            # pair-1 scatters issue last: their transfers queue behind the
            # x stream on Q1 (~35us) — still ~45us before conv pair 1
            # reads wblk1.
            hop1(2, nc.sync)
            hop1(3, nc.sync)

            # ---- stage B: per-pair conv, chunk-outer / tap-inner ----
            for p in range(PAIRS):
                xp3 = xps[p].rearrange("p (h w) -> p h w", w=WP)
                for g in range(8):
                    os = opool.tile([128, 2048], f16, name=f"os{p}_{g}", tag="os")
                    for j in range(4):
                        h0 = (g * 4 + j) * 4
                        po = ptile([128, 512], name=f"po{p}_{g}_{j}")
                        for t in range(NT):
                            kh, kw = divmod(t, 3)
                            nc.tensor.matmul(
                                po[:],
                                wblk[p][:, t * 128 : (t + 1) * 128],
                                xp3[:, h0 + kh : h0 + kh + 4, kw : kw + W],
                                start=(t == 0),
                                stop=(t == NT - 1),
                            )
                        if j % 2 == 0:
                            nc.vector.tensor_scalar_add(
                                os[:, j * 512 : (j + 1) * 512], po[:], cb_sb
                            )
                        else:
                            nc.scalar.add(
                                os[:, j * 512 : (j + 1) * 512], po[:], cb_sb
                            )
                        if p == PAIRS - 1 and g == 7:
                            # final group: store per 4-row chunk right
                            # after its bias-copy so the LAST transfer is
                            # only 128KB (~0.5us) — the single [1024:2048]
                            # tail store's 256KB transfer held the
                            # teardown barrier ~2.1us past the last bias.
                            # j<2 on ACT, j>=2 on the idle SP queue.
                            eng = nc.scalar if j < 2 else nc.sync
                            eng.dma_start(
                                out=out_d.ap()[
                                    2 * p : 2 * p + 2,
                                    :,
                                    16 * g + 4 * j : 16 * g + 4 * j + 4,
                                    :,
                                ],
                                in_=os[:, j * 512 : (j + 1) * 512],
                            )
                    if p == PAIRS - 1 and g == 7:
                        pass  # stored per-chunk above
                    else:
                        nc.scalar.dma_start(
                            out=out_d.ap()[
                                2 * p : 2 * p + 2, :, 16 * g : 16 * g + 16, :
                            ],
                            in_=os[:],
                        )
                    if p == 0 and g == 6:
                        # pair-1 weight fanout, slotted into DVE/ACT slack
                        # mid pair-0 conv (hop1(2)/(3) have long since
                        # drained; wblk1 is needed ~35us later)
                        fanout(2, nc.vector)
                        fanout(3, nc.scalar)

            # tail junk: the MID monitor re-throttles the PE to 4/8 at
            # conv_end+3.4us, putting the Tensor engine's 51 teardown
            # semaphore resets (the longest epilogue pole) at half clock
            # (~115ns each vs ~67ns warm). A short burst keeps the clock
            # 8/8 into the reset run; sized to finish before the final
            # store's completion gates the teardown barrier.
            for w in range(8):
                pw = ptile([128, 512], name=f"tail{w}")
                nc.tensor.matmul(
                    pw[:], ht8b[:], w2s[:, 0:512], start=True, stop=True
                )

    nc.compile()
    return nc


def _get_nc():
    if "nc" not in _CACHE:
        _CACHE["nc"] = _build()
    return _CACHE["nc"]


def _prep(x, c, conv_w, conv_b, mlp_w1, mlp_b1, mlp_w2, mlp_b2):
    x = np.asarray(x, dtype=np.float32)
    c = np.asarray(c, dtype=np.float32)
    conv_w = np.asarray(conv_w, dtype=np.float32)
    conv_b = np.asarray(conv_b, dtype=np.float32)
    mlp_w1 = np.asarray(mlp_w1, dtype=np.float32)
    mlp_b1 = np.asarray(mlp_b1, dtype=np.float32)
    mlp_w2 = np.asarray(mlp_w2, dtype=np.float32)
    mlp_b2 = np.asarray(mlp_b2, dtype=np.float32)

    # padded fp16 x, flattened spatial
    xsp = np.zeros((B, CIN, HP, WP), dtype=np.float16)
    xsp[:, :, 1 : HP - 1, 1 : WP - 1] = x.astype(np.float16)
    xsp = xsp.reshape(B, CIN, HP * WP)

    # w2p[k, (ci, t, co)] = mlp_w2[k, co*576 + ci*9 + t]
    # row 16 = (mlp_b2 + conv_w), same permutation -> adj == full weight
    w2p = mlp_w2.reshape(MH, COUT, CIN, NT).transpose(0, 2, 3, 1).reshape(MH, WTOT)
    b2p = mlp_b2.reshape(COUT, CIN, NT).transpose(1, 2, 0)
    cwp = conv_w.reshape(COUT, CIN, NT).transpose(1, 2, 0)  # [ci, t, co]
    row16 = (b2p + cwp).reshape(1, WTOT)
    w2p = np.concatenate([w2p, row16], axis=0)  # [17, 36864]
    # 4-group pack: rows 32g..32g+17 carry cols [9216g, 9216(g+1))
    w2pk = np.zeros((128, GCOL), dtype=np.float16)
    for g in range(4):
        w2pk[32 * g : 32 * g + K2] = w2p[:, GCOL * g : GCOL * (g + 1)].astype(
            np.float16
        )

    # packed consts [128, 66] f32 (core-invariant part): cols 0-31 c'T
    # tiled 8x, 32-63 w1' zero-padded to 32, 64 b1 tiled per k-group,
    # 65 conv_b x2
    cstb = np.zeros((128, 66), dtype=np.float32)
    cstb[:CL, 32 : 32 + MH] = mlp_w1
    cstb[CL, 32 + MH] = 1.0
    for g in range(4):
        cstb[32 * g : 32 * g + MH, 64] = mlp_b1
    cstb[:, 65] = np.tile(conv_b, 2)

    in_maps = []
    for i in range(NCORES):
        sl = slice(i * BPC, (i + 1) * BPC)
        cst = cstb.copy()
        cst[:CL, 0:32] = np.tile(c[sl].T, (1, 8))
        cst[CL, 0:32] = 1.0
        in_maps.append(
            {
                "xsp": np.ascontiguousarray(xsp[sl]),
                "w2p": w2pk,
                "cst": cst,
            }
        )
    return in_maps


def _run(inputs, trace=False):
    from concourse.bass_utils import run_bass_kernel_spmd

    nc = _get_nc()
    in_maps = _prep(**inputs)
    res = run_bass_kernel_spmd(
        nc, in_maps, core_ids=list(range(NCORES)), trace=trace
    )
    out = np.concatenate(
        [res.results[i]["out"].astype(np.float32) for i in range(NCORES)], axis=0
    )
    return out, res


def kernel(**inputs):
    out, _ = _run(inputs, trace=False)
    return out



# revision 66
# speedup vs baseline: 1.0210x; 1.0210x over previous
"""Trainium2 Bass kernel for conditional-adjustment conv (CAConv), fp16.

Per sample b: h = relu(c[b] @ mlp_w1 + mlp_b1); adj = h @ mlp_w2 + mlp_b2;
w[b] = conv_w + adj.reshape(Co,Ci,3,3); out[b] = conv2d(x[b], w[b], pad=1) + conv_b.

Sharding: data-parallel over batch, 4 samples per core on 8 cores (SPMD).

All heavy matmuls in fp16 (full PE rate); psum accumulation stays fp32, so
rel err ~5e-4 << the 2e-2 budget. The host pre-casts padded x and the packed
w2 to fp16 and the kernel returns fp16 output (halves HBM traffic both
ways); the host casts back to fp32.

Per-core device kernel:
  Stage A (weight gen): four col-group fp32 matmuls (M=32 via
  zero-padded w1', tile_position=(0,32g)) compute the MLP hidden state
  for all 4 samples directly as a [128, 32] psum tile; one DVE fused
  add-bias+relu produces hT8 fp16 (col m = sample m%4, replicated at
  partition offsets 0/32/64/96 to match the packed w2). w2 is
  host-permuted to (ci, t, co) column order, packed in 4 k-groups
  [128, 9216] (group g = ci 16g..16g+16, rows 32g..32g+17), with
  mlp_b2 + conv_w folded into ones-row 16. For each 512-col chunk, 4
  matmuls with tile_position=(32g, 32g) write M=32 rows each so one
  psum tile [128, 512] is fully covered -> full-width DVE/ACT copies
  (fp32->fp16) into the partition-grouped adj4 [128, 9216] (row
  32g + 4r + b = sample b, ci group g, replica r).
  Weight placement is a two-hop scatter: (1) a 64-descriptor DMA per
  sample (partition-stride-32 source) into the compact staging tile
  wst[ci + 64*half, (t, co)]; (2) a same-partition strided DVE/ACT copy
  fans each half out onto the diagonal blocks of the per-pair
  block-diag tile wblk[ci + 64*half, t*128 + 64*half + co] (off-diag
  zeros from a DVE memset). This avoids the 576 tiny 128B descriptors
  per sample a direct scatter would need (measured ~16us of DMA-engine
  serialization gating conv start).
  Stage B (conv): host-padded fp16 x (130x130) for a sample pair lives
  as [ci(2 samples), h, w] across 128 partitions. Chunk-outer/
  tap-inner: each output chunk po[128, 512] (2 samples x 64 co
  partitions; 4 h-rows x 128 w free) accumulates its 9 shift-tap K=128
  fp16 matmuls back-to-back at the PE's 1 col/cycle peak (~218ns/MM),
  then its bias-copy (alternating DVE/ACT) fires immediately. Psum
  banks rotate through an explicit 8-tag round-robin ring (the pool's
  own slot picker reuses banks ~2 tiles apart, each reuse a ~1.3us PE
  stall on the copy drain). One output DMA per 16 h-rows; the final
  group's store is split so the tail is short. Plain junk matmuls pad
  the PE between stage A and conv so the HAM clock gate stays at
  full rate (col-tiled matmuls don't register as PE-busy to it).

  All loads ride the SP HWDGE queue (Q1) in consumption order (cst,
  w2 in graduated chunks, the 19-row xp0 head, pair-0 hop1 scatters,
  then the x backlog, then pair-1 scatters); output stores use the ACT
  queue (Q10). The arbiter gives Q1 strict priority, so anything
  parked on other queues starves while Q1 streams — pair-1's scatters
  exploit this deliberately (needed ~45us later).

  HAM (PE clock gate) discipline, measured on silicon: the PE boots at
  4/8 duty (1.2GHz) and un-throttles only after ~3.4us of sustained
  FULL-ARRAY activity; tile-positioned quartets and small (K=17/M=32)
  matmuls are invisible to BOTH monitors, so the original quartet
  stage A ran entirely cold and conv paid a ~4us cold ramp. Stage A's
  body is therefore a block-diagonal ht8b [128,128] stationary driving
  ONE plain K=128 matmul per 512-col chunk (same math as the quartet,
  one instruction, HAM-visible), and ~24 plain junk matmuls bridge the
  scatter/fanout hole so the MID monitor (re-throttles after ~3.4us
  without qualifying activity) never fires before conv.

  Stage A's tail is copy-chain limited: each chunk's psum->adj4 copy
  is split in half across DVE and ACT running simultaneously
  (whole-copy alternation paced the body at ~460-690ns/chunk).
  hop1 scatter descriptors (64 x 1152B, SBUF->SBUF) serialize on a
  small engine set at ~2us/DMA regardless of queue: pair-0's two
  scatters run alone on Q1 at the body's end, pair-1's trail the x
  backlog, and their fanouts are emitted mid-conv-pair-0 so they never
  head-block the conv bias-copies.

  Measured: 158.8us (HW exec) in the chip's fast state (conv matmuls
  at 217ns/512-col = full 2.4GHz streaming rate, no HAM events
  mid-kernel); the board alternates run-to-run into a power-throttled
  state (264ns spacing, everything including DMA ~21% slower) where
  the same kernel measures ~190us. Conv start at ~25.4us
  (fast-equivalent) vs 28.1 for the staged baseline; remaining fixed
  overhead: ~6us NEFF prologue (excluded from exec_time), ~3us first-
  DMA cold latency, ~6.5us w2 stream (gates stage A), ~5us scatter+
  fanout, ~10us teardown (full semaphore-file reset + drains).
"""

import sys

if "/opt/trn_rl_repo" not in sys.path:
    sys.path.insert(0, "/opt/trn_rl_repo")

import numpy as np

B = 32
NCORES = 8
BPC = B // NCORES          # samples per core = 4
PAIRS = BPC // 2           # sample pairs per core = 2
CIN = COUT = 64
H = W = 128
HP = WP = 130              # padded dims
KH = KW = 3
NT = KH * KW               # taps = 9
CL = 8                     # c length
CL1 = CL + 1               # + ones row
MH = 16                    # mlp hidden
K2 = MH + 1                # mlp hidden + ones row
WTOT = NT * CIN * COUT     # 36864 weights per sample
GCOL = WTOT // 4           # 9216 cols per packed w2 group
XCH = 5                    # x chunks per pair
XCHE = (HP * WP) // XCH    # 3380 elems per chunk (26 padded rows)

_CACHE = {}


def _build():
    import concourse.bass as bass
    import concourse.mybir as mybir
    import concourse.tile as tile
    from concourse import bacc
    from concourse.tile_rust import add_dep_helper

    f32 = mybir.dt.float32
    f16 = mybir.dt.float16
    AF = mybir.ActivationFunctionType

    nc = bacc.Bacc("TRN2", target_bir_lowering=False, debug=False)

    xs_d = nc.dram_tensor("xsp", [BPC, CIN, HP * WP], f16, kind="ExternalInput")
    w2_d = nc.dram_tensor("w2p", [128, GCOL], f16, kind="ExternalInput")
    cst_d = nc.dram_tensor("cst", [128, 66], f32, kind="ExternalInput")
    out_d = nc.dram_tensor("out", [BPC, COUT, H, W], f16, kind="ExternalOutput")

    with tile.TileContext(nc) as tc:
        with (
            tc.tile_pool(name="consts", bufs=1) as consts,
            tc.tile_pool(name="adjpool", bufs=1) as adjpool,
            tc.tile_pool(name="xpool", bufs=2) as xpool,
            tc.tile_pool(name="opool", bufs=8) as opool,
            tc.tile_pool(name="pspool", bufs=1, space=bass.MemorySpace.PSUM) as ps,
        ):
            # ---- consts then packed w2 on the SP queue (gate stage A);
            # GPSIMD is avoided entirely: its DSPs take ~10us of ucode
            # load/drain before their first op ----
            # cst: cols 0-31 c'T tiled 8x (rows 0-8, ones row 8, col m =
            # sample m%4), 32-63 w1' zero-padded to M=32 (rows 0-8),
            # 64 b1 tiled at partition offsets 0/32/64/96, 65 conv_b x2
            cst = consts.tile([128, 66], f32)
            nc.sync.dma_start(out=cst[:], in_=cst_d.ap())
            # w2 128-row packed (rows 32g..32g+17 = group g): 2.36MB vs
            # 1.25MB dense, but a dense [17, *] load has only 17
            # descriptors per DMA and the engines assign per-descriptor
            # round-robin within a DMA — measured: the whole dense load
            # serialized onto ONE engine at ~55us. 128 descriptors/DMA
            # spread over all 16 engines beat the byte savings.
            w2s = consts.tile([128, GCOL], f16, name="w2s")
            # graduated chunks: small leading chunks so stage A starts
            # early and never sees a multi-us burst wait; larger tail
            # chunks keep the issue count down (each dma_start costs
            # ~0.6us of SP time)
            w2cuts = [0, 512, 1536, 3072, 4608, 6912, GCOL]
            for c0, c1 in zip(w2cuts, w2cuts[1:]):
                nc.sync.dma_start(
                    out=w2s[:, c0:c1], in_=w2_d.ap()[:, c0:c1]
                )
            ct_sb = cst[0:CL1, 0:32]
            w1_sb = cst[0:CL1, 32:64]
            b1_sb = cst[:, 64:65]
            cb_sb = cst[:, 65:66]

            # psum bank ring: explicit round-robin via 8 single-buffer
            # tags — the pool's own slot picker reuses banks as little
            # as 1 tile apart, each reuse a ~1.3us PE stall on the copy
            # drain.
            psk = [0]

            def ptile(shape, name=None):
                # four [128, 1024] two-bank pair-tiles, explicit round-
                # robin; callers take a leading slice. Stage A accumulates
                # two 512-col chunks per tile so ONE copy drains both;
                # conv po uses cols [0:512] (ring-4, ~5us reuse slack).
                t = ps.tile(
                    [128, 1024], f32, tag=f"pp{psk[0] % 4}", bufs=1, name=name
                )
                psk[0] += 1
                return t[:, 0 : shape[1]]

            # ---- stage A head: hT8 via one matmul quartet, then fused
            # bias+relu into the DIAGONAL blocks of ht8b [128, 128]
            # (zeros elsewhere). ht8b block-diag makes the stage A body a
            # PLAIN K=128 matmul per 512-col chunk: one LDW+MM instead of
            # a 4x tile_position quartet, and (critically) plain full-K
            # matmuls register as "busy" to the PE_HAM clock gate, so the
            # PE un-throttles 4/8 -> 8/8 DURING stage A instead of 3.9us
            # into conv. (Measured: tile-positioned quartets are invisible
            # to both HAM monitors — stage A ran entirely at 1.2GHz and
            # the MID monitor even re-throttled mid-kernel.) ----
            ph32 = ptile([128, 32], name="ph32")
            for g in range(4):
                nc.tensor.matmul(
                    ph32[32 * g : 32 * g + 32, :], w1_sb, ct_sb,
                    start=True, stop=True, tile_position=(0, 32 * g),
                )
            ht8b = consts.tile([128, 128], f16, name="ht8b")
            nc.vector.memset(ht8b[:], 0.0)
            for g in range(4):
                nc.vector.tensor_scalar(
                    out=ht8b[32 * g : 32 * g + K2, 32 * g : 32 * g + 32],
                    in0=ph32[32 * g : 32 * g + K2, :],
                    scalar1=cst[32 * g : 32 * g + K2, 64:65], scalar2=0.0,
                    op0=mybir.AluOpType.add, op1=mybir.AluOpType.max,
                )

            # early junk burst: plain full-K matmuls fill the w2-DMA wait
            # window right after ht8b is ready, so the HAM's SHORT monitor
            # fires ~3.4us in (measured: deterministic at ~14us) instead
            # of the run-variable 16-24us when warm-up relied on the
            # body's spaced matmuls alone.
            for w in range(100, 107):
                pw = ptile([128, 512], name=f"warm{w}")
                nc.tensor.matmul(
                    pw[:], ht8b[:], w2s[:, 0:512], start=True, stop=True
                )

            # ---- bulk x loads, all on Q1 (SP). The DMA arbiter gives
            # Q1 strict priority, so parking x on another queue starves
            # that queue instead of helping; the lever that works is
            # ORDER within the Q1 ring: only the two xp0 chunks conv
            # needs first go ahead of the hop1 scatters, the rest after.
            xps = []
            for p in range(PAIRS):
                xp = xpool.tile([128, HP * WP], f16, name=f"xp{p}", tag="xp")
                xps.append(xp)

            # custom row cuts: a small head chunk (rows 0-18, all that
            # conv group g=0 reads) lands fast right behind w2 on Q1; the
            # rest follows the hop1 scatters and stays far ahead of the
            # conv's row consumption.
            XCUTS = [0, 2470, 6110, 9750, 13390, HP * WP]

            def load_x(p, ks, after=()):
                for k in ks:
                    e0, e1 = XCUTS[k], XCUTS[k + 1]
                    inst = nc.sync.dma_start(
                        out=xps[p][:, e0:e1],
                        in_=xs_d.ap()[2 * p : 2 * p + 2, :, e0:e1],
                    )
                    for a in after:
                        add_dep_helper(
                            inst.ins, a.ins, sync=True, reason="x after hop1"
                        )

            load_x(0, range(1))

            # per-pair block-diag weights; off-diag zeros via DVE memset
            # (DVE is idle early; GPSIMD wouldn't wake up for ~10us).
            # Emitted after the hT8 op so they don't head-block it in
            # the DVE FIFO.
            wblk = []
            for p in range(PAIRS):
                wb = consts.tile([128, NT * 128], f16, name=f"wblk{p}", tag=f"wblk{p}")
                nc.vector.memset(wb[:], 0.0)
                wblk.append(wb)

            # ---- stage A body: adj4[32g + 4r + b, c] = sample b's weight
            # for flat col 9216g + c ((ci,t,co) order), r = 0..7 replicas ----
            adj4 = adjpool.tile([128, GCOL], f16, name="adj4")
            pair = None
            for m in range(GCOL // 512):
                j = m * 512
                if m % 2 == 0:
                    pair = ptile([128, 1024], name=f"pa{m}")
                half = (m % 2) * 512
                nc.tensor.matmul(
                    pair[:, half : half + 512], ht8b[:], w2s[:, j : j + 512],
                    start=True, stop=True,
                )
                if m % 2 == 1:
                    j0 = j - 512
                    # one copy drains both chunks (9 copies total vs 18:
                    # the ~500ns per-op issue+sem overhead paced adj4
                    # ~7us behind the w2 stream); the final pair splits
                    # across both engines (critical path to hop1)
                    if m == GCOL // 512 - 1:
                        nc.vector.tensor_copy(
                            adj4[:, j0 : j0 + 512], pair[:, 0:512]
                        )
                        nc.scalar.copy(
                            adj4[:, j0 + 512 : j0 + 1024], pair[:, 512:1024]
                        )
                    elif (m // 2) % 2 == 0:
                        nc.vector.tensor_copy(adj4[:, j0 : j0 + 1024], pair[:])
                    else:
                        nc.scalar.copy(adj4[:, j0 : j0 + 1024], pair[:])

            # two-hop weight placement (see module docstring)
            wst = []
            for p in range(PAIRS):
                wstp = consts.tile([128, NT * 64], f16, name=f"wst{p}", tag=f"wst{p}")
                wst.append(wstp)
            # hop1: each dma_start costs ~0.6us of ENGINE time (flat,
            # regardless of descriptor count), so the four scatters are
            # spread across SP/SP/DVE/ACT to issue in parallel the moment
            # adj4's last copy retires.
            def hop1(b, eng):
                p, half = divmod(b, 2)
                q = half * 64
                # read sample b's replica r=2b (row 32g + 4r + b = 32g+9b):
                # source partitions 9b, 9b+32, .. land on a DISJOINT DMA
                # engine quartet per sample (engine ~ partition//8), so
                # the four scatters' transfers run in parallel instead of
                # serializing on engines {0,4,8,12}.
                return eng.dma_start(
                    out=wst[p][q : q + 64, :], in_=adj4[9 * b : 128 : 32, :]
                )

            def fanout(b, eng):
                p, half = divmod(b, 2)
                q = half * 64
                dst = wblk[p][q : q + 64, :].rearrange(
                    "p (t co) -> p t co", co=128
                )[:, :, q : q + 64]
                src = wst[p][q : q + 64, :].rearrange("p (t co) -> p t co", co=64)
                if eng is nc.vector:
                    eng.tensor_copy(dst, src)
                else:
                    eng.copy(dst, src)

            # junk bridge over the hop1/fanout hole: plain full-K matmuls
            # (results never read) hold the HAM's busy monitor so the
            # clock stays 8/8 into conv. Sized to OVERLAP the whole
            # scatter+fanout handoff (~8us): an undersized bridge leaves
            # a >3.4us idle gap where the MID monitor re-throttles and
            # conv restarts cold (~3us penalty, run-to-run luck); the
            # overshoot cost is bounded by conv waiting on the last junk
            # (~equal to the handoff length either way).
            for w in range(26):
                pw = ptile([128, 512], name=f"warm{w}")
                nc.tensor.matmul(
                    pw[:], ht8b[:], w2s[:, 0:512], start=True, stop=True
                )

            # pair-0 scatters first: hop1(0) on SP/Q1, hop1(1) on ACT/Q10
            # (empty until the first conv store) — disjoint queues AND,
            # via the replica trick, disjoint engine quartets, so their
            # issues and transfers run fully in parallel.
            h_a = hop1(0, nc.sync)
            h_b = hop1(1, nc.scalar)
            fanout(0, nc.vector)
            fanout(1, nc.scalar)
            # fanout(2)/(3) are emitted mid-way through conv pair 0 (see
            # below) so they don't head-block the conv's DVE/ACT
            # bias-copies.
            # x backlog after the pair-0 scatters (Q1 FIFO order).
            load_x(0, range(1, XCH), after=(h_b,))
            load_x(1, range(XCH), after=(h_b,))
            # pair-1 scatters issue last: their transfers queue behind the
            # x stream on Q1 (~35us) — still ~45us before conv pair 1
            # reads wblk1.
            hop1(2, nc.sync)
            hop1(3, nc.sync)

            # ---- stage B: per-pair conv, chunk-outer / tap-inner ----
            for p in range(PAIRS):
                xp3 = xps[p].rearrange("p (h w) -> p h w", w=WP)
                for g in range(8):
                    os = opool.tile([128, 2048], f16, name=f"os{p}_{g}", tag="os")
                    for j in range(4):
                        h0 = (g * 4 + j) * 4
                        po = ptile([128, 512], name=f"po{p}_{g}_{j}")
                        for t in range(NT):
                            kh, kw = divmod(t, 3)
                            nc.tensor.matmul(
                                po[:],
                                wblk[p][:, t * 128 : (t + 1) * 128],
                                xp3[:, h0 + kh : h0 + kh + 4, kw : kw + W],
                                start=(t == 0),
                                stop=(t == NT - 1),
                            )
                        if j % 2 == 0:
                            nc.vector.tensor_scalar_add(
                                os[:, j * 512 : (j + 1) * 512], po[:], cb_sb
                            )
                        else:
                            nc.scalar.add(
                                os[:, j * 512 : (j + 1) * 512], po[:], cb_sb
                            )
                        if p == PAIRS - 1 and g == 7:
                            # final group: store per 4-row chunk right
                            # after its bias-copy so the LAST transfer is
                            # only 128KB (~0.5us) — the single [1024:2048]
                            # tail store's 256KB transfer held the
                            # teardown barrier ~2.1us past the last bias.
                            # j<2 on ACT, j>=2 on the idle SP queue.
                            eng = nc.scalar if j < 2 else nc.sync
                            eng.dma_start(
                                out=out_d.ap()[
                                    2 * p : 2 * p + 2,
                                    :,
                                    16 * g + 4 * j : 16 * g + 4 * j + 4,
                                    :,
                                ],
                                in_=os[:, j * 512 : (j + 1) * 512],
                            )
                    if p == PAIRS - 1 and g == 7:
                        pass  # stored per-chunk above
                    else:
                        nc.scalar.dma_start(
                            out=out_d.ap()[
                                2 * p : 2 * p + 2, :, 16 * g : 16 * g + 16, :
                            ],
                            in_=os[:],
                        )
                    if p == 0 and g == 6:
                        # pair-1 weight fanout, slotted into DVE/ACT slack
                        # mid pair-0 conv (hop1(2)/(3) have long since
                        # drained; wblk1 is needed ~35us later)
                        fanout(2, nc.vector)
                        fanout(3, nc.scalar)

            # tail junk: the MID monitor re-throttles the PE to 4/8 at
            # conv_end+3.4us, putting the Tensor engine's 51 teardown
            # semaphore resets (the longest epilogue pole) at half clock
            # (~115ns each vs ~67ns warm). A short burst keeps the clock
            # 8/8 into the reset run; sized to finish before the final
            # store's completion gates the teardown barrier.
            for w in range(8):
                pw = ptile([128, 512], name=f"tail{w}")
                nc.tensor.matmul(
                    pw[:], ht8b[:], w2s[:, 0:512], start=True, stop=True
                )

    nc.compile()
    return nc


def _get_nc():
    if "nc" not in _CACHE:
        _CACHE["nc"] = _build()
    return _CACHE["nc"]


def _prep(x, c, conv_w, conv_b, mlp_w1, mlp_b1, mlp_w2, mlp_b2):
    x = np.asarray(x, dtype=np.float32)
    c = np.asarray(c, dtype=np.float32)
    conv_w = np.asarray(conv_w, dtype=np.float32)
    conv_b = np.asarray(conv_b, dtype=np.float32)
    mlp_w1 = np.asarray(mlp_w1, dtype=np.float32)
    mlp_b1 = np.asarray(mlp_b1, dtype=np.float32)
    mlp_w2 = np.asarray(mlp_w2, dtype=np.float32)
    mlp_b2 = np.asarray(mlp_b2, dtype=np.float32)

    # padded fp16 x, flattened spatial
    xsp = np.zeros((B, CIN, HP, WP), dtype=np.float16)
    xsp[:, :, 1 : HP - 1, 1 : WP - 1] = x.astype(np.float16)
    xsp = xsp.reshape(B, CIN, HP * WP)

    # w2p[k, (ci, t, co)] = mlp_w2[k, co*576 + ci*9 + t]
    # row 16 = (mlp_b2 + conv_w), same permutation -> adj == full weight
    w2p = mlp_w2.reshape(MH, COUT, CIN, NT).transpose(0, 2, 3, 1).reshape(MH, WTOT)
    b2p = mlp_b2.reshape(COUT, CIN, NT).transpose(1, 2, 0)
    cwp = conv_w.reshape(COUT, CIN, NT).transpose(1, 2, 0)  # [ci, t, co]
    row16 = (b2p + cwp).reshape(1, WTOT)
    w2p = np.concatenate([w2p, row16], axis=0)  # [17, 36864]
    # 4-group pack: rows 32g..32g+17 carry cols [9216g, 9216(g+1))
    w2pk = np.zeros((128, GCOL), dtype=np.float16)
    for g in range(4):
        w2pk[32 * g : 32 * g + K2] = w2p[:, GCOL * g : GCOL * (g + 1)].astype(
            np.float16
        )

    # packed consts [128, 66] f32 (core-invariant part): cols 0-31 c'T
    # tiled 8x, 32-63 w1' zero-padded to 32, 64 b1 tiled per k-group,
    # 65 conv_b x2
    cstb = np.zeros((128, 66), dtype=np.float32)
    cstb[:CL, 32 : 32 + MH] = mlp_w1
    cstb[CL, 32 + MH] = 1.0
    for g in range(4):
        cstb[32 * g : 32 * g + MH, 64] = mlp_b1
    cstb[:, 65] = np.tile(conv_b, 2)

    in_maps = []
    for i in range(NCORES):
        sl = slice(i * BPC, (i + 1) * BPC)
        cst = cstb.copy()
        cst[:CL, 0:32] = np.tile(c[sl].T, (1, 8))
        cst[CL, 0:32] = 1.0
        in_maps.append(
            {
                "xsp": np.ascontiguousarray(xsp[sl]),
                "w2p": w2pk,
                "cst": cst,
            }
        )
    return in_maps


def _run(inputs, trace=False):
    from concourse.bass_utils import run_bass_kernel_spmd

    nc = _get_nc()
    in_maps = _prep(**inputs)
    res = run_bass_kernel_spmd(
        nc, in_maps, core_ids=list(range(NCORES)), trace=trace
    )
    out = np.concatenate(
        [res.results[i]["out"].astype(np.float32) for i in range(NCORES)], axis=0
    )
    return out, res


def kernel(**inputs):
    out, _ = _run(inputs, trace=False)
    return out



# revision 69
# speedup vs baseline: 1.0212x; 1.0002x over previous
"""Trainium2 Bass kernel for conditional-adjustment conv (CAConv), fp16.

Per sample b: h = relu(c[b] @ mlp_w1 + mlp_b1); adj = h @ mlp_w2 + mlp_b2;
w[b] = conv_w + adj.reshape(Co,Ci,3,3); out[b] = conv2d(x[b], w[b], pad=1) + conv_b.

Sharding: data-parallel over batch, 4 samples per core on 8 cores (SPMD).

All heavy matmuls in fp16 (full PE rate); psum accumulation stays fp32, so
rel err ~5e-4 << the 2e-2 budget. The host pre-casts padded x and the packed
w2 to fp16 and the kernel returns fp16 output (halves HBM traffic both
ways); the host casts back to fp32.

Per-core device kernel:
  Stage A (weight gen): four col-group fp32 matmuls (M=32 via
  zero-padded w1', tile_position=(0,32g)) compute the MLP hidden state
  for all 4 samples directly as a [128, 32] psum tile; one DVE fused
  add-bias+relu produces hT8 fp16 (col m = sample m%4, replicated at
  partition offsets 0/32/64/96 to match the packed w2). w2 is
  host-permuted to (ci, t, co) column order, packed in 4 k-groups
  [128, 9216] (group g = ci 16g..16g+16, rows 32g..32g+17), with
  mlp_b2 + conv_w folded into ones-row 16. For each 512-col chunk, 4
  matmuls with tile_position=(32g, 32g) write M=32 rows each so one
  psum tile [128, 512] is fully covered -> full-width DVE/ACT copies
  (fp32->fp16) into the partition-grouped adj4 [128, 9216] (row
  32g + 4r + b = sample b, ci group g, replica r).
  Weight placement is a two-hop scatter: (1) a 64-descriptor DMA per
  sample (partition-stride-32 source) into the compact staging tile
  wst[ci + 64*half, (t, co)]; (2) a same-partition strided DVE/ACT copy
  fans each half out onto the diagonal blocks of the per-pair
  block-diag tile wblk[ci + 64*half, t*128 + 64*half + co] (off-diag
  zeros from a DVE memset). This avoids the 576 tiny 128B descriptors
  per sample a direct scatter would need (measured ~16us of DMA-engine
  serialization gating conv start).
  Stage B (conv): host-padded fp16 x (130x130) for a sample pair lives
  as [ci(2 samples), h, w] across 128 partitions. Chunk-outer/
  tap-inner: each output chunk po[128, 512] (2 samples x 64 co
  partitions; 4 h-rows x 128 w free) accumulates its 9 shift-tap K=128
  fp16 matmuls back-to-back at the PE's 1 col/cycle peak (~218ns/MM),
  then its bias-copy (alternating DVE/ACT) fires immediately. Psum
  banks rotate through an explicit 8-tag round-robin ring (the pool's
  own slot picker reuses banks ~2 tiles apart, each reuse a ~1.3us PE
  stall on the copy drain). One output DMA per 16 h-rows; the final
  group's store is split so the tail is short. Plain junk matmuls pad
  the PE between stage A and conv so the HAM clock gate stays at
  full rate (col-tiled matmuls don't register as PE-busy to it).

  All loads ride the SP HWDGE queue (Q1) in consumption order (cst,
  w2 in graduated chunks, the 19-row xp0 head, pair-0 hop1 scatters,
  then the x backlog, then pair-1 scatters); output stores use the ACT
  queue (Q10). The arbiter gives Q1 strict priority, so anything
  parked on other queues starves while Q1 streams — pair-1's scatters
  exploit this deliberately (needed ~45us later).

  HAM (PE clock gate) discipline, measured on silicon: the PE boots at
  4/8 duty (1.2GHz) and un-throttles only after ~3.4us of sustained
  FULL-ARRAY activity; tile-positioned quartets and small (K=17/M=32)
  matmuls are invisible to BOTH monitors, so the original quartet
  stage A ran entirely cold and conv paid a ~4us cold ramp. Stage A's
  body is therefore a block-diagonal ht8b [128,128] stationary driving
  ONE plain K=128 matmul per 512-col chunk (same math as the quartet,
  one instruction, HAM-visible), and ~24 plain junk matmuls bridge the
  scatter/fanout hole so the MID monitor (re-throttles after ~3.4us
  without qualifying activity) never fires before conv.

  Stage A's tail is copy-chain limited: each chunk's psum->adj4 copy
  is split in half across DVE and ACT running simultaneously
  (whole-copy alternation paced the body at ~460-690ns/chunk).
  hop1 scatter descriptors (64 x 1152B, SBUF->SBUF) serialize on a
  small engine set at ~2us/DMA regardless of queue: pair-0's two
  scatters run alone on Q1 at the body's end, pair-1's trail the x
  backlog, and their fanouts are emitted mid-conv-pair-0 so they never
  head-block the conv bias-copies.

  Measured: 158.8us (HW exec) in the chip's fast state (conv matmuls
  at 217ns/512-col = full 2.4GHz streaming rate, no HAM events
  mid-kernel); the board alternates run-to-run into a power-throttled
  state (264ns spacing, everything including DMA ~21% slower) where
  the same kernel measures ~190us. Conv start at ~25.4us
  (fast-equivalent) vs 28.1 for the staged baseline; remaining fixed
  overhead: ~6us NEFF prologue (excluded from exec_time), ~3us first-
  DMA cold latency, ~6.5us w2 stream (gates stage A), ~5us scatter+
  fanout, ~10us teardown (full semaphore-file reset + drains).
"""

import sys

if "/opt/trn_rl_repo" not in sys.path:
    sys.path.insert(0, "/opt/trn_rl_repo")

import numpy as np

B = 32
NCORES = 8
BPC = B // NCORES          # samples per core = 4
PAIRS = BPC // 2           # sample pairs per core = 2
CIN = COUT = 64
H = W = 128
HP = WP = 130              # padded dims
KH = KW = 3
NT = KH * KW               # taps = 9
CL = 8                     # c length
CL1 = CL + 1               # + ones row
MH = 16                    # mlp hidden
K2 = MH + 1                # mlp hidden + ones row
WTOT = NT * CIN * COUT     # 36864 weights per sample
GCOL = WTOT // 4           # 9216 cols per packed w2 group
XCH = 5                    # x chunks per pair
XCHE = (HP * WP) // XCH    # 3380 elems per chunk (26 padded rows)

_CACHE = {}


def _build():
    import concourse.bass as bass
    import concourse.mybir as mybir
    import concourse.tile as tile
    from concourse import bacc
    from concourse.tile_rust import add_dep_helper

    f32 = mybir.dt.float32
    f16 = mybir.dt.float16
    AF = mybir.ActivationFunctionType

    nc = bacc.Bacc("TRN2", target_bir_lowering=False, debug=False)

    xs_d = nc.dram_tensor("xsp", [BPC, CIN, HP * WP], f16, kind="ExternalInput")
    w2_d = nc.dram_tensor("w2p", [128, GCOL], f16, kind="ExternalInput")
    cst_d = nc.dram_tensor("cst", [128, 66], f32, kind="ExternalInput")
    out_d = nc.dram_tensor("out", [BPC, COUT, H, W], f16, kind="ExternalOutput")

    with tile.TileContext(nc) as tc:
        with (
            tc.tile_pool(name="consts", bufs=1) as consts,
            tc.tile_pool(name="adjpool", bufs=1) as adjpool,
            tc.tile_pool(name="xpool", bufs=2) as xpool,
            tc.tile_pool(name="opool", bufs=8) as opool,
            tc.tile_pool(name="pspool", bufs=1, space=bass.MemorySpace.PSUM) as ps,
        ):
            # ---- consts then packed w2 on the SP queue (gate stage A);
            # GPSIMD is avoided entirely: its DSPs take ~10us of ucode
            # load/drain before their first op ----
            # cst: cols 0-31 c'T tiled 8x (rows 0-8, ones row 8, col m =
            # sample m%4), 32-63 w1' zero-padded to M=32 (rows 0-8),
            # 64 b1 tiled at partition offsets 0/32/64/96, 65 conv_b x2
            cst = consts.tile([128, 66], f32)
            nc.sync.dma_start(out=cst[:], in_=cst_d.ap())
            # w2 128-row packed (rows 32g..32g+17 = group g): 2.36MB vs
            # 1.25MB dense, but a dense [17, *] load has only 17
            # descriptors per DMA and the engines assign per-descriptor
            # round-robin within a DMA — measured: the whole dense load
            # serialized onto ONE engine at ~55us. 128 descriptors/DMA
            # spread over all 16 engines beat the byte savings.
            w2s = consts.tile([128, GCOL], f16, name="w2s")
            # graduated chunks: small leading chunks so stage A starts
            # early and never sees a multi-us burst wait; larger tail
            # chunks keep the issue count down (each dma_start costs
            # ~0.6us of SP time)
            w2cuts = [0, 512, 1536, 3072, 4608, 6912, GCOL]
            for c0, c1 in zip(w2cuts, w2cuts[1:]):
                nc.sync.dma_start(
                    out=w2s[:, c0:c1], in_=w2_d.ap()[:, c0:c1]
                )
            ct_sb = cst[0:CL1, 0:32]
            w1_sb = cst[0:CL1, 32:64]
            b1_sb = cst[:, 64:65]
            cb_sb = cst[:, 65:66]

            # psum bank ring: explicit round-robin via 8 single-buffer
            # tags — the pool's own slot picker reuses banks as little
            # as 1 tile apart, each reuse a ~1.3us PE stall on the copy
            # drain.
            psk = [0]

            def ptile(shape, name=None):
                t = ps.tile(shape, f32, tag=f"ps{psk[0] % 8}", bufs=1, name=name)
                psk[0] += 1
                return t

            # ---- stage A head: hT8 via one matmul quartet, then fused
            # bias+relu into the DIAGONAL blocks of ht8b [128, 128]
            # (zeros elsewhere). ht8b block-diag makes the stage A body a
            # PLAIN K=128 matmul per 512-col chunk: one LDW+MM instead of
            # a 4x tile_position quartet, and (critically) plain full-K
            # matmuls register as "busy" to the PE_HAM clock gate, so the
            # PE un-throttles 4/8 -> 8/8 DURING stage A instead of 3.9us
            # into conv. (Measured: tile-positioned quartets are invisible
            # to both HAM monitors — stage A ran entirely at 1.2GHz and
            # the MID monitor even re-throttled mid-kernel.) ----
            ph32 = ptile([128, 32], name="ph32")
            for g in range(4):
                nc.tensor.matmul(
                    ph32[32 * g : 32 * g + 32, :], w1_sb, ct_sb,
                    start=True, stop=True, tile_position=(0, 32 * g),
                )
            ht8b = consts.tile([128, 128], f16, name="ht8b")
            nc.vector.memset(ht8b[:], 0.0)
            for g in range(4):
                nc.vector.tensor_scalar(
                    out=ht8b[32 * g : 32 * g + K2, 32 * g : 32 * g + 32],
                    in0=ph32[32 * g : 32 * g + K2, :],
                    scalar1=cst[32 * g : 32 * g + K2, 64:65], scalar2=0.0,
                    op0=mybir.AluOpType.add, op1=mybir.AluOpType.max,
                )

            # early junk burst: plain full-K matmuls fill the w2-DMA wait
            # window right after ht8b is ready, so the HAM's SHORT monitor
            # fires ~3.4us in (measured: deterministic at ~14us) instead
            # of the run-variable 16-24us when warm-up relied on the
            # body's spaced matmuls alone.
            for w in range(100, 107):
                pw = ptile([128, 512], name=f"warm{w}")
                nc.tensor.matmul(
                    pw[:], ht8b[:], w2s[:, 0:512], start=True, stop=True
                )

            # ---- bulk x loads, all on Q1 (SP). The DMA arbiter gives
            # Q1 strict priority, so parking x on another queue starves
            # that queue instead of helping; the lever that works is
            # ORDER within the Q1 ring: only the two xp0 chunks conv
            # needs first go ahead of the hop1 scatters, the rest after.
            xps = []
            for p in range(PAIRS):
                xp = xpool.tile([128, HP * WP], f16, name=f"xp{p}", tag="xp")
                xps.append(xp)

            # custom row cuts: a small head chunk (rows 0-18, all that
            # conv group g=0 reads) lands fast right behind w2 on Q1; the
            # rest follows the hop1 scatters and stays far ahead of the
            # conv's row consumption.
            XCUTS = [0, 2470, 6110, 9750, 13390, HP * WP]

            def load_x(p, ks, after=()):
                for k in ks:
                    e0, e1 = XCUTS[k], XCUTS[k + 1]
                    inst = nc.sync.dma_start(
                        out=xps[p][:, e0:e1],
                        in_=xs_d.ap()[2 * p : 2 * p + 2, :, e0:e1],
                    )
                    for a in after:
                        add_dep_helper(
                            inst.ins, a.ins, sync=True, reason="x after hop1"
                        )

            load_x(0, range(1))

            # per-pair block-diag weights; off-diag zeros via DVE memset
            # (DVE is idle early; GPSIMD wouldn't wake up for ~10us).
            # Emitted after the hT8 op so they don't head-block it in
            # the DVE FIFO.
            wblk = []
            for p in range(PAIRS):
                wb = consts.tile([128, NT * 128], f16, name=f"wblk{p}", tag=f"wblk{p}")
                nc.vector.memset(wb[:], 0.0)
                wblk.append(wb)

            # ---- stage A body: adj4[32g + 4r + b, c] = sample b's weight
            # for flat col 9216g + c ((ci,t,co) order), r = 0..7 replicas ----
            adj4 = adjpool.tile([128, GCOL], f16, name="adj4")
            for m in range(GCOL // 512):
                j = m * 512
                pa = ptile([128, 512])
                nc.tensor.matmul(
                    pa[:], ht8b[:], w2s[:, j : j + 512], start=True, stop=True
                )
                # whole-512 copies alternating DVE/ACT: 9 ops per engine
                # amortizes the ~200-300ns per-op issue+sem overhead
                # better than half-splits (18 ops/engine), keeping the
                # ring's copy drain close to the w2 stream's 360ns/chunk
                # pace. Only the final chunk (critical path to hop1)
                # splits across both engines.
                if m == GCOL // 512 - 1:
                    nc.vector.tensor_copy(adj4[:, j : j + 256], pa[:, 0:256])
                    nc.scalar.copy(adj4[:, j + 256 : j + 512], pa[:, 256:512])
                elif m % 2 == 0:
                    nc.vector.tensor_copy(adj4[:, j : j + 512], pa[:])
                else:
                    nc.scalar.copy(adj4[:, j : j + 512], pa[:])

            # two-hop weight placement (see module docstring)
            wst = []
            for p in range(PAIRS):
                wstp = consts.tile([128, NT * 64], f16, name=f"wst{p}", tag=f"wst{p}")
                wst.append(wstp)
            # hop1: each dma_start costs ~0.6us of ENGINE time (flat,
            # regardless of descriptor count), so the four scatters are
            # spread across SP/SP/DVE/ACT to issue in parallel the moment
            # adj4's last copy retires.
            def hop1(b, eng):
                p, half = divmod(b, 2)
                q = half * 64
                # read sample b's replica r=2b (row 32g + 4r + b = 32g+9b):
                # source partitions 9b, 9b+32, .. land on a DISJOINT DMA
                # engine quartet per sample (engine ~ partition//8), so
                # the four scatters' transfers run in parallel instead of
                # serializing on engines {0,4,8,12}.
                return eng.dma_start(
                    out=wst[p][q : q + 64, :], in_=adj4[9 * b : 128 : 32, :]
                )

            def fanout(b, eng, t0=0, t1=NT):
                p, half = divmod(b, 2)
                q = half * 64
                dst = wblk[p][q : q + 64, :].rearrange(
                    "p (t co) -> p t co", co=128
                )[:, t0:t1, q : q + 64]
                src = wst[p][q : q + 64, :].rearrange(
                    "p (t co) -> p t co", co=64
                )[:, t0:t1, :]
                if eng is nc.vector:
                    eng.tensor_copy(dst, src)
                else:
                    eng.copy(dst, src)

            # junk bridge over the hop1/fanout hole: plain full-K matmuls
            # (results never read) hold the HAM's busy monitor so the
            # clock stays 8/8 into conv. Sized to OVERLAP the whole
            # scatter+fanout handoff (~8us): an undersized bridge leaves
            # a >3.4us idle gap where the MID monitor re-throttles and
            # conv restarts cold (~3us penalty, run-to-run luck); the
            # overshoot cost is bounded by conv waiting on the last junk
            # (~equal to the handoff length either way).
            for w in range(26):
                pw = ptile([128, 512], name=f"warm{w}")
                nc.tensor.matmul(
                    pw[:], ht8b[:], w2s[:, 0:512], start=True, stop=True
                )

            # pair-0 scatters first: hop1(0) on SP/Q1, hop1(1) on ACT/Q10
            # (empty until the first conv store) — disjoint queues AND,
            # via the replica trick, disjoint engine quartets, so their
            # issues and transfers run fully in parallel.
            h_a = hop1(0, nc.sync)
            h_b = hop1(1, nc.scalar)
            # pair-0 fanouts split by tap range across BOTH engines: the
            # whole-half fanout was a single ~0.8us op serializing after
            # the gating scatter; tap-halves run ~0.45us in parallel.
            fanout(0, nc.vector, 0, 5)
            fanout(0, nc.scalar, 5, NT)
            fanout(1, nc.scalar, 0, 5)
            fanout(1, nc.vector, 5, NT)
            # fanout(2)/(3) are emitted mid-way through conv pair 0 (see
            # below) so they don't head-block the conv's DVE/ACT
            # bias-copies.
            # x backlog after the pair-0 scatters (Q1 FIFO order).
            load_x(0, range(1, XCH), after=(h_b,))
            load_x(1, range(XCH), after=(h_b,))
            # pair-1 scatters issue last: their transfers queue behind the
            # x stream on Q1 (~35us) — still ~45us before conv pair 1
            # reads wblk1.
            hop1(2, nc.sync)
            hop1(3, nc.sync)

            # ---- stage B: per-pair conv, chunk-outer / tap-inner ----
            for p in range(PAIRS):
                xp3 = xps[p].rearrange("p (h w) -> p h w", w=WP)
                for g in range(8):
                    os = opool.tile([128, 2048], f16, name=f"os{p}_{g}", tag="os")
                    for j in range(4):
                        h0 = (g * 4 + j) * 4
                        po = ptile([128, 512], name=f"po{p}_{g}_{j}")
                        for t in range(NT):
                            kh, kw = divmod(t, 3)
                            nc.tensor.matmul(
                                po[:],
                                wblk[p][:, t * 128 : (t + 1) * 128],
                                xp3[:, h0 + kh : h0 + kh + 4, kw : kw + W],
                                start=(t == 0),
                                stop=(t == NT - 1),
                            )
                        if j % 2 == 0:
                            nc.vector.tensor_scalar_add(
                                os[:, j * 512 : (j + 1) * 512], po[:], cb_sb
                            )
                        else:
                            nc.scalar.add(
                                os[:, j * 512 : (j + 1) * 512], po[:], cb_sb
                            )
                        if p == PAIRS - 1 and g == 7:
                            # final group: store per 4-row chunk right
                            # after its bias-copy so the LAST transfer is
                            # only 128KB (~0.5us) — the single [1024:2048]
                            # tail store's 256KB transfer held the
                            # teardown barrier ~2.1us past the last bias.
                            # j<2 on ACT, j>=2 on the idle SP queue.
                            eng = nc.scalar if j < 2 else nc.sync
                            eng.dma_start(
                                out=out_d.ap()[
                                    2 * p : 2 * p + 2,
                                    :,
                                    16 * g + 4 * j : 16 * g + 4 * j + 4,
                                    :,
                                ],
                                in_=os[:, j * 512 : (j + 1) * 512],
                            )
                    if p == PAIRS - 1 and g == 7:
                        pass  # stored per-chunk above
                    else:
                        nc.scalar.dma_start(
                            out=out_d.ap()[
                                2 * p : 2 * p + 2, :, 16 * g : 16 * g + 16, :
                            ],
                            in_=os[:],
                        )
                    if p == 0 and g == 6:
                        # pair-1 weight fanout, slotted into DVE/ACT slack
                        # mid pair-0 conv (hop1(2)/(3) have long since
                        # drained; wblk1 is needed ~35us later)
                        fanout(2, nc.vector)
                        fanout(3, nc.scalar)

            # tail junk: the MID monitor re-throttles the PE to 4/8 at
            # conv_end+3.4us, putting the Tensor engine's 51 teardown
            # semaphore resets (the longest epilogue pole) at half clock
            # (~115ns each vs ~67ns warm). A short burst keeps the clock
            # 8/8 into the reset run; sized to finish before the final
            # store's completion gates the teardown barrier.
            for w in range(8):
                pw = ptile([128, 512], name=f"tail{w}")
                nc.tensor.matmul(
                    pw[:], ht8b[:], w2s[:, 0:512], start=True, stop=True
                )

    nc.compile()
    return nc


def _get_nc():
    if "nc" not in _CACHE:
        _CACHE["nc"] = _build()
    return _CACHE["nc"]


def _prep(x, c, conv_w, conv_b, mlp_w1, mlp_b1, mlp_w2, mlp_b2):
    x = np.asarray(x, dtype=np.float32)
    c = np.asarray(c, dtype=np.float32)
    conv_w = np.asarray(conv_w, dtype=np.float32)
    conv_b = np.asarray(conv_b, dtype=np.float32)
    mlp_w1 = np.asarray(mlp_w1, dtype=np.float32)
    mlp_b1 = np.asarray(mlp_b1, dtype=np.float32)
    mlp_w2 = np.asarray(mlp_w2, dtype=np.float32)
    mlp_b2 = np.asarray(mlp_b2, dtype=np.float32)

    # padded fp16 x, flattened spatial
    xsp = np.zeros((B, CIN, HP, WP), dtype=np.float16)
    xsp[:, :, 1 : HP - 1, 1 : WP - 1] = x.astype(np.float16)
    xsp = xsp.reshape(B, CIN, HP * WP)

    # w2p[k, (ci, t, co)] = mlp_w2[k, co*576 + ci*9 + t]
    # row 16 = (mlp_b2 + conv_w), same permutation -> adj == full weight
    w2p = mlp_w2.reshape(MH, COUT, CIN, NT).transpose(0, 2, 3, 1).reshape(MH, WTOT)
    b2p = mlp_b2.reshape(COUT, CIN, NT).transpose(1, 2, 0)
    cwp = conv_w.reshape(COUT, CIN, NT).transpose(1, 2, 0)  # [ci, t, co]
    row16 = (b2p + cwp).reshape(1, WTOT)
    w2p = np.concatenate([w2p, row16], axis=0)  # [17, 36864]
    # 4-group pack: rows 32g..32g+17 carry cols [9216g, 9216(g+1))
    w2pk = np.zeros((128, GCOL), dtype=np.float16)
    for g in range(4):
        w2pk[32 * g : 32 * g + K2] = w2p[:, GCOL * g : GCOL * (g + 1)].astype(
            np.float16
        )

    # packed consts [128, 66] f32 (core-invariant part): cols 0-31 c'T
    # tiled 8x, 32-63 w1' zero-padded to 32, 64 b1 tiled per k-group,
    # 65 conv_b x2
    cstb = np.zeros((128, 66), dtype=np.float32)
    cstb[:CL, 32 : 32 + MH] = mlp_w1
    cstb[CL, 32 + MH] = 1.0
    for g in range(4):
        cstb[32 * g : 32 * g + MH, 64] = mlp_b1
    cstb[:, 65] = np.tile(conv_b, 2)

    in_maps = []
    for i in range(NCORES):
        sl = slice(i * BPC, (i + 1) * BPC)
        cst = cstb.copy()
        cst[:CL, 0:32] = np.tile(c[sl].T, (1, 8))
        cst[CL, 0:32] = 1.0
        in_maps.append(
            {
                "xsp": np.ascontiguousarray(xsp[sl]),
                "w2p": w2pk,
                "cst": cst,
            }
        )
    return in_maps


def _run(inputs, trace=False):
    from concourse.bass_utils import run_bass_kernel_spmd

    nc = _get_nc()
    in_maps = _prep(**inputs)
    res = run_bass_kernel_spmd(
        nc, in_maps, core_ids=list(range(NCORES)), trace=trace
    )
    out = np.concatenate(
        [res.results[i]["out"].astype(np.float32) for i in range(NCORES)], axis=0
    )
    return out, res


def kernel(**inputs):
    out, _ = _run(inputs, trace=False)
    return out

